# revision 2
# baseline (speedup 1.0000x reference)
"""Trainium2 Bass kernel for nn_BitwiseWavenetBlock (10-layer gated WaveNet block).

Strategy: data-parallel over batch (16 batches -> 8 cores x 2 sequential).
Each core runs the full 10-layer block on [128, 16384] activations resident in
SBUF. Dilated k=2 convs are pairs of PSUM-accumulated 128x128 matmuls against
shifted windows of an fp16 activation buffer with zeroed margins. Weight
gating (W * sigmoid(G)), per-channel scale s and biases are folded on the host.

Engine split per 1024-col chunk: PE 12 matmuls; ACT bias-add+fp16-cast of
filter/gate PSUM; GpSimd gating multiply + fp16 recast of the residual;
DVE fused (psum + bias) + master accumulations for skip/resid (fp32 masters).
"""

import sys

sys.path.insert(0, "/opt/trn_rl_repo")

import numpy as np
import ml_dtypes

import concourse.bass as bass
import concourse.bacc as bacc
import concourse.mybir as mybir
import concourse.tile as tile
from concourse.bass_utils import run_bass_kernel_spmd

F32 = mybir.dt.float32
F16 = mybir.dt.float16
AF = mybir.ActivationFunctionType
ALU = mybir.AluOpType

N_CORES = 8
LAYERS = 10
C = 128          # channels (= partitions)
L = 16384        # sequence length
B = 16           # total batch
BPC = B // N_CORES  # batches per core (sequential)
CW = 1024        # chunk width (2 PSUM banks)
NCH = L // CW
MG = 256         # xb margin (>= max shift 2^8)
NH = CW // 512   # matmul halves per chunk


def _build_nc(bpc=BPC, l_len=L, layers=LAYERS):
    nch = l_len // CW
    nc = bacc.Bacc(None)
    x_in = nc.declare_dram_parameter("x", [bpc, C, l_len], F32, isOutput=False)
    wts_in = nc.declare_dram_parameter("wts", [C, layers * 6 * C], F16, isOutput=False)
    bias_in = nc.declare_dram_parameter("biases", [C, layers * 4], F32, isOutput=False)
    resid_out = nc.declare_dram_parameter("resid", [bpc, C, l_len], F32, isOutput=True)
    skip_out = nc.declare_dram_parameter("skip", [bpc, C, l_len], F32, isOutput=True)

    with tile.TileContext(nc) as tc:
        with (
            tc.tile_pool(name="constp", bufs=1) as constp,
            tc.tile_pool(name="masterp", bufs=1) as masterp,
            tc.tile_pool(name="workp", bufs=3) as workp,
            tc.tile_pool(name="psump", bufs=1, space="PSUM") as psump,
        ):
            wts = constp.tile([C, layers * 6 * C], F16)
            nc.sync.dma_start(wts[:], wts_in[:])
            biases = constp.tile([C, layers * 4], F32)
            nc.sync.dma_start(biases[:], bias_in[:])

            resid_m = masterp.tile([C, l_len], F32)
            skip_m = masterp.tile([C, l_len], F32)
            xb = masterp.tile([C, l_len + 2 * MG], F16)
            nc.vector.memset(xb[:], 0.0)

            def wmat(l, j):
                return wts[:, (l * 6 + j) * C : (l * 6 + j + 1) * C]

            def bvec(l, j):
                return biases[:, l * 4 + j : l * 4 + j + 1]

            def cast_chunk(c):
                nc.gpsimd.tensor_copy(
                    xb[:, MG + c * CW : MG + (c + 1) * CW],
                    resid_m[:, c * CW : (c + 1) * CW],
                )

            for b in range(bpc):
                for c in range(nch):
                    cs = slice(c * CW, (c + 1) * CW)
                    nc.sync.dma_start(resid_m[:, cs], x_in[b, :, cs])
                    cast_chunk(c)
                for l in range(layers):
                    s0 = 1 if l == 0 else 2 ** (l - 1)
                    s1 = 0 if l == 0 else 2 ** (l - 1)
                    pend_cast = None
                    for c in range(nch):
                        f_ps = psump.tile([C, CW], F32, tag="f", name="f_ps")
                        g_ps = psump.tile([C, CW], F32, tag="g", name="g_ps")
                        # weight-grouped order: each stationary matrix streams
                        # the full chunk before switching
                        for j, sh, start in (
                            (0, -s0, True),
                            (1, s1, False),
                            (2, -s0, True),
                            (3, s1, False),
                        ):
                            ps = f_ps if j < 2 else g_ps
                            for hh in range(NH):
                                col = c * CW + hh * 512
                                nc.tensor.matmul(
                                    ps[:, hh * 512 : (hh + 1) * 512],
                                    wmat(l, j),
                                    xb[:, MG + col + sh : MG + col + sh + 512],
                                    start=start,
                                    stop=not start,
                                )
                        # lagged fp16 recast of previous chunk (next layer's input):
                        # emitted after this chunk's f/g matmuls so the backward
                        # tap of chunk c has already read layer-l values.
                        if pend_cast is not None and l < layers - 1:
                            cast_chunk(pend_cast)
                        pend_cast = c

                        fb = workp.tile([C, CW], F16, tag="fb", name="fb")
                        gb = workp.tile([C, CW], F16, tag="gb", name="gb")
                        nc.scalar.activation(fb[:], f_ps[:], AF.Identity, bias=bvec(l, 0))
                        nc.scalar.activation(gb[:], g_ps[:], AF.Identity, bias=bvec(l, 1))
                        h = workp.tile([C, CW], F16, tag="h", name="h")
                        nc.gpsimd.tensor_mul(h[:], fb[:], gb[:])

                        s_ps = psump.tile([C, CW], F32, tag="s", name="s_ps")
                        r_ps = psump.tile([C, CW], F32, tag="r", name="r_ps")
                        for hh in range(NH):
                            hs = slice(hh * 512, (hh + 1) * 512)
                            nc.tensor.matmul(s_ps[:, hs], wmat(l, 4), h[:, hs], start=True, stop=True)
                        for hh in range(NH):
                            hs = slice(hh * 512, (hh + 1) * 512)
                            nc.tensor.matmul(r_ps[:, hs], wmat(l, 5), h[:, hs], start=True, stop=True)

                        cs = slice(c * CW, (c + 1) * CW)
                        if l == 0:
                            # skip master is uninitialized: write, don't accumulate
                            nc.vector.tensor_scalar_add(skip_m[:, cs], s_ps[:], bvec(l, 2))
                        else:
                            nc.vector.scalar_tensor_tensor(
                                skip_m[:, cs], s_ps[:], bvec(l, 2), skip_m[:, cs],
                                op0=ALU.add, op1=ALU.add,
                            )
                        nc.vector.scalar_tensor_tensor(
                            resid_m[:, cs], r_ps[:], bvec(l, 3), resid_m[:, cs],
                            op0=ALU.add, op1=ALU.add,
                        )
                        if l == layers - 1:
                            nc.sync.dma_start(resid_out[b, :, cs], resid_m[:, cs])
                            nc.sync.dma_start(skip_out[b, :, cs], skip_m[:, cs])
                    if pend_cast is not None and l < layers - 1:
                        cast_chunk(pend_cast)

    nc.finalize()
    return nc


def _sigmoid(x):
    return 1.0 / (1.0 + np.exp(-x))


def _fold(W, G, b, s):
    W = np.asarray(W, np.float32)
    G = np.asarray(G, np.float32)
    b = np.asarray(b, np.float32)
    s = np.asarray(s, np.float32)
    Weff = s[:, :, None, None] * W * _sigmoid(G)
    return Weff.astype(np.float32), (s * b).astype(np.float32)


def _prep_params(Wf, Gf, bf, sf, Wg, Gg, bg, sg, Wr, Gr, br, sr, Ws, Gs, bs, ss,
                 layers=LAYERS):
    Wf_e, bf_e = _fold(Wf, Gf, bf, sf)
    Wg_e, bg_e = _fold(Wg, Gg, bg, sg)
    Wr_e, br_e = _fold(Wr, Gr, br, sr)
    Ws_e, bs_e = _fold(Ws, Gs, bs, ss)

    # wts_host[p, l*6+j, m] = lhsT_j[p, m] = W'_j[m, p] (stationary = W'^T)
    wts_host = np.zeros((C, layers * 6, C), np.float32)
    bias_host = np.zeros((C, layers * 4), np.float32)
    for l in range(layers):
        mats = [Wf_e[l, :, :, 0], Wf_e[l, :, :, 1],
                Wg_e[l, :, :, 0], Wg_e[l, :, :, 1],
                Ws_e[l, :, :, 0], Wr_e[l, :, :, 0]]
        for j, m in enumerate(mats):
            wts_host[:, l * 6 + j, :] = m.T
        bias_host[:, l * 4 + 0] = bf_e[l]
        bias_host[:, l * 4 + 1] = bg_e[l]
        bias_host[:, l * 4 + 2] = bs_e[l]
        bias_host[:, l * 4 + 3] = br_e[l]
    wts_host = wts_host.reshape(C, layers * 6 * C).astype(np.float16)
    return wts_host, bias_host


_NC_CACHE = {}


def kernel(x, Wf, Gf, bf, sf, Wg, Gg, bg, sg, Wr, Gr, br, sr, Ws, Gs, bs, ss):
    x = np.asarray(x, np.float32)
    wts_host, bias_host = _prep_params(Wf, Gf, bf, sf, Wg, Gg, bg, sg,
                                       Wr, Gr, br, sr, Ws, Gs, bs, ss)
    if "nc" not in _NC_CACHE:
        _NC_CACHE["nc"] = _build_nc()
    nc = _NC_CACHE["nc"]

    in_maps = [
        {"x": np.ascontiguousarray(x[c * BPC : (c + 1) * BPC]),
         "wts": wts_host, "biases": bias_host}
        for c in range(N_CORES)
    ]
    res = run_bass_kernel_spmd(nc, in_maps, list(range(N_CORES)))
    resid = np.concatenate([res.results[c]["resid"] for c in range(N_CORES)], axis=0)
    skip = np.concatenate([res.results[c]["skip"] for c in range(N_CORES)], axis=0)
    return resid, skip
